# revision 1
# baseline (speedup 1.0000x reference)
"""SupJSD / ContrastiveLossPlus loss kernel for 8 Trainium2 NeuronCores.

Single pass over the [3N, D] data. Per 128-row tile:
  ss_i  = sum_d x^2           (DVE scalar_tensor_tensor, accum)
  s16_i = 16/sqrt(ss)         (ACT: exp(-0.5*ln(ss)+ln16), batched per group)
  lg    = ln(s16*x + 1e-30)   (ACT Ln with per-partition scale)  [= ln(16*p)]
  u_i   = sum_d x*lg          (DVE stt accum, into column 256 of the tile)
  A     = (cls==lab)*s16      (DVE fused tensor_scalar)  [one-hot * 16/||x||]
  psum += A^T @ [x | u]       (PE fp32 matmul, N=257)
Host combines the per-class [80,257] accumulators in float64:
  loss = 0.01/D * sum_c (E'_c - sum_d seg*ln(16*mix)) / counts_c
"""

import numpy as np

N_CORES = 8
N, D, C = 65536, 256, 80
R = 3 * N // N_CORES          # rows per core = 24576
T = R // 128                  # tiles per core = 192
G = 16                        # tiles per small-op group
LOG16 = float(np.log(16.0))

_cache = {}


def _build_nc():
    from contextlib import ExitStack

    import concourse.tile as tile
    from concourse import bacc, mybir

    F32 = mybir.dt.float32
    A = mybir.AluOpType
    ACTF = mybir.ActivationFunctionType

    nc = bacc.Bacc("TRN2", target_bir_lowering=False, debug=False,
                   num_devices=N_CORES)
    xin = nc.dram_tensor("xin", [R, D], F32, kind="ExternalInput").ap()
    labt = nc.dram_tensor("labt", [128, T], F32, kind="ExternalInput").ap()
    cls = nc.dram_tensor("cls", [128, C], F32, kind="ExternalInput").ap()
    out = nc.dram_tensor("acc", [C, D + 1], F32, kind="ExternalOutput").ap()

    with tile.TileContext(nc) as tc, ExitStack() as ctx:
        cpool = ctx.enter_context(tc.tile_pool(name="consts", bufs=1))
        xpool = ctx.enter_context(tc.tile_pool(name="x", bufs=2 * G + 4))
        lgpool = ctx.enter_context(tc.tile_pool(name="lg", bufs=3))
        jpool = ctx.enter_context(tc.tile_pool(name="junk", bufs=2))
        apool = ctx.enter_context(tc.tile_pool(name="amat", bufs=3))
        spool = ctx.enter_context(tc.tile_pool(name="small", bufs=2))
        opool = ctx.enter_context(tc.tile_pool(name="out", bufs=1))
        pspool = ctx.enter_context(tc.tile_pool(name="ps", bufs=1, space="PSUM"))

        clst = cpool.tile([128, C], F32)
        nc.sync.dma_start(clst[:], cls[:])
        labs = cpool.tile([128, T], F32)
        nc.sync.dma_start(labs[:], labt[:])
        c_ln16 = cpool.tile([128, 1], F32)
        nc.vector.memset(c_ln16[:], LOG16)
        c_tiny = cpool.tile([128, 1], F32)
        nc.vector.memset(c_tiny[:], 1e-30)

        ps = pspool.tile([C, D + 1], F32)
        junk1 = jpool.tile([128, D], F32, tag="junk")
        junk2 = jpool.tile([128, D], F32, tag="junk")

        for g in range(T // G):
            xts = []
            ssg = spool.tile([128, G], F32, tag="ssg")
            for j in range(G):
                k = g * G + j
                xu = xpool.tile([128, D + 1], F32, tag="xu")
                nc.sync.dma_start(xu[:, 0:D], xin[k * 128:(k + 1) * 128, :])
                nc.vector.scalar_tensor_tensor(
                    junk1[:], xu[:, 0:D], 1.0, xu[:, 0:D], A.mult, A.mult,
                    accum_out=ssg[:, j:j + 1])
                xts.append(xu)
            # s16 = exp(-0.5*ln(max(ss,1e-24)) + ln16) = 16/sqrt(ss)
            lssg = spool.tile([128, G], F32, tag="lssg")
            nc.vector.tensor_scalar(lssg[:], ssg[:], 1e-24, None, A.max)
            nc.scalar.activation(lssg[:], lssg[:], ACTF.Ln)
            s16g = spool.tile([128, G], F32, tag="s16g")
            nc.scalar.activation(s16g[:], lssg[:], ACTF.Exp,
                                 bias=c_ln16[:], scale=-0.5)
            for j in range(G):
                k = g * G + j
                xu = xts[j]
                s16 = s16g[:, j:j + 1]
                lg = lgpool.tile([128, D], F32, tag="lg")
                nc.scalar.activation(lg[:], xu[:, 0:D], ACTF.Ln,
                                     bias=c_tiny[:], scale=s16)
                nc.vector.scalar_tensor_tensor(
                    junk2[:], xu[:, 0:D], 1.0, lg[:], A.mult, A.mult,
                    accum_out=xu[:, D:D + 1])
                amat = apool.tile([128, C], F32, tag="amat")
                nc.vector.tensor_scalar(amat[:], clst[:], labs[:, k:k + 1],
                                        s16, A.is_equal, A.mult)
                nc.tensor.matmul(ps[:], amat[:], xu[:],
                                 start=(k == 0), stop=(k == T - 1))

        acc = opool.tile([C, D + 1], F32)
        nc.vector.tensor_copy(acc[:], ps[:])
        nc.sync.dma_start(out[:], acc[:])
    nc.compile()
    return nc


def _get_nc():
    if "nc" not in _cache:
        _cache["nc"] = _build_nc()
    return _cache["nc"]


def kernel(logits_clean, logits_aug1, logits_aug2, labels):
    import os

    from concourse.bass_utils import run_bass_kernel_spmd

    x3 = np.concatenate(
        [np.asarray(logits_clean, dtype=np.float32),
         np.asarray(logits_aug1, dtype=np.float32),
         np.asarray(logits_aug2, dtype=np.float32)], axis=0)
    lab1 = np.asarray(labels).astype(np.int64)
    lab3 = np.concatenate([lab1, lab1, lab1])

    cls = np.ascontiguousarray(
        np.broadcast_to(np.arange(C, dtype=np.float32), (128, C)))
    in_maps = []
    for c in range(N_CORES):
        sl = slice(c * R, (c + 1) * R)
        in_maps.append({
            "xin": np.ascontiguousarray(x3[sl]),
            "labt": np.ascontiguousarray(
                lab3[sl].reshape(T, 128).T.astype(np.float32)),
            "cls": cls,
        })

    nc = _get_nc()
    trace = bool(int(os.environ.get("KERNEL_TRACE", "0")))
    kw = {}
    if trace:
        kw = dict(trace=True, tmpdir=os.environ.get("KERNEL_TRACE_DIR"))
    br = run_bass_kernel_spmd(nc, in_maps, list(range(N_CORES)), **kw)
    _cache["last_results"] = br

    acc = np.zeros((C, D + 1), np.float64)
    for c in range(N_CORES):
        acc += br.results[c]["acc"].astype(np.float64)

    seg = acc[:, :D] / 16.0            # sum_{i in c} p_i  (per dim)
    Ep = acc[:, D] / 16.0              # sum_{i in c} sum_d p*ln(16p)
    counts = np.bincount(lab3, minlength=C).astype(np.float64)
    mix = seg / np.maximum(counts, 1.0)[:, None]
    lm16 = np.log(np.maximum(mix, 1e-7)) + np.log(16.0)
    num = Ep - (seg * lm16).sum(1)
    loss = np.where(counts > 0, num / np.maximum(counts, 1.0), 0.0).sum() / D
    return np.float32(0.01 * loss)



# revision 5
# speedup vs baseline: 1.2308x; 1.2308x over previous
"""SupJSD / ContrastiveLossPlus loss kernel for 8 Trainium2 NeuronCores.

Math: p_i = x_i/||x_i||, loss = 0.01 * sum_c (1/n_c)[sum_{i in c} sum_d
p ln p - sum_d S_c * ln(clip(S_c/n_c))] / D, with S_c = sum_{i in c} p_i.
Using ln p = ln x - 0.5*ln(ss_i) (ss = sum x^2), per class we need only:
  A_x[c,d] = sum_i 1hot*s16*x        (16*S_c)
  A_z[c,d] = sum_i 1hot*s16*z        (z = x*ln x)
  B[c,d]   = sum_i 1hot*(s16*lnss)*x
since sum_{i in c} sum_d p ln p = (sum_d A_z - 0.5*sum_d B)/16.
All three are PE matmuls with per-tile stationaries built from one-hot
rows scaled per-partition; no per-row u accumulation is needed.

Engine split per 128x2048 group (8 tiles): DMA loads bf16 x
(partition-major, 4KB contiguous per partition); row sums-of-squares
alternate between DVE (TT x*x + 8x tensor_scalar-accum) and ACT
(8x Square+accum) groups to balance busy time; ACT does one batched
Ln over the group; DVE builds z and the two stationaries; PE runs 3
accumulating matmuls per tile. One activation-table load (set with
ln+exp+square) is pinned up front. Host combines the [80,768] per-core
accumulators in float64.
"""

import numpy as np

N_CORES = 8
N, D, C = 65536, 256, 80
R = 3 * N // N_CORES          # rows per core = 24576
T = R // 128                  # tiles per core = 192
G = 8                         # tiles per group
NG = T // G                   # groups = 24
LOG16 = float(np.log(16.0))

_cache = {}


def _build_nc():
    from contextlib import ExitStack

    import concourse.tile as tile
    from concourse import bacc, mybir

    F32 = mybir.dt.float32
    BF16 = mybir.dt.bfloat16
    A = mybir.AluOpType
    ACTF = mybir.ActivationFunctionType

    nc = bacc.Bacc("TRN2", target_bir_lowering=False, debug=False,
                   num_devices=N_CORES)
    # Partition-major: xin[p, g*2048 + j*256 + c] = x[(g*8+j)*128 + p, c]
    xin = nc.dram_tensor("xin", [128, T * 256], BF16, kind="ExternalInput").ap()
    labt = nc.dram_tensor("labt", [128, T], F32, kind="ExternalInput").ap()
    cls = nc.dram_tensor("cls", [128, C], BF16, kind="ExternalInput").ap()
    out = nc.dram_tensor("acc", [C, 768], F32, kind="ExternalOutput").ap()

    with tile.TileContext(nc) as tc, ExitStack() as ctx:
        cpool = ctx.enter_context(tc.tile_pool(name="consts", bufs=1))
        xpool = ctx.enter_context(tc.tile_pool(name="x", bufs=3))
        qpool = ctx.enter_context(tc.tile_pool(name="q", bufs=2))
        lpool = ctx.enter_context(tc.tile_pool(name="lnx", bufs=2))
        zpool = ctx.enter_context(tc.tile_pool(name="z", bufs=3))
        jpool = ctx.enter_context(tc.tile_pool(name="junk", bufs=1))
        spool = ctx.enter_context(tc.tile_pool(name="small", bufs=3))
        apool = ctx.enter_context(tc.tile_pool(name="amat", bufs=4))
        bpool = ctx.enter_context(tc.tile_pool(name="bmat", bufs=4))
        opool = ctx.enter_context(tc.tile_pool(name="out", bufs=1))
        psa = ctx.enter_context(tc.tile_pool(name="psa", bufs=1, space="PSUM"))
        psz = ctx.enter_context(tc.tile_pool(name="psz", bufs=1, space="PSUM"))
        psb = ctx.enter_context(tc.tile_pool(name="psb", bufs=1, space="PSUM"))

        clst = cpool.tile([128, C], BF16)
        nc.sync.dma_start(clst[:], cls[:])
        labs = cpool.tile([128, T], F32)
        nc.sync.dma_start(labs[:], labt[:])
        c_tiny = cpool.tile([128, 1], F32)
        nc.vector.memset(c_tiny[:], 1e-30)
        c_ln16 = cpool.tile([128, 1], F32)
        nc.vector.memset(c_ln16[:], LOG16)

        # Pin the ln+exp+square table once: avoids per-group table thrash.
        nc.scalar.add_instruction(mybir.InstLoadActFuncSet(
            name=nc.get_next_instruction_name(), act_func_set_id=6,
            ins=[], outs=[]))

        psA = psa.tile([C, 256], F32)
        psZ = psz.tile([C, 256], F32)
        psB = psb.tile([C, 256], F32)
        junk_d = jpool.tile([128, 256], BF16, tag="jd")
        junk_a = jpool.tile([128, 256], BF16, tag="ja")

        for g in range(NG):
            xg = xpool.tile([128, G * 256], BF16, tag="x")
            nc.sync.dma_start(xg[:], xin[:, g * G * 256:(g + 1) * G * 256])

            ssg = spool.tile([128, G], F32, tag="ss")
            if g % 2 == 1:
                for j in range(G):
                    nc.scalar.activation(
                        junk_a[:], xg[:, j * 256:(j + 1) * 256], ACTF.Square,
                        accum_out=ssg[:, j:j + 1])
            else:
                qg = qpool.tile([128, G * 256], BF16, tag="q")
                nc.vector.tensor_tensor(qg[:], xg[:], xg[:], A.mult)
                for j in range(G):
                    nc.vector.tensor_scalar(
                        junk_d[:], qg[:, j * 256:(j + 1) * 256], 1.0, 0.0,
                        A.mult, A.add, accum_out=ssg[:, j:j + 1])

            lnssg = spool.tile([128, G], F32, tag="lnss")
            nc.scalar.activation(lnssg[:], ssg[:], ACTF.Ln)
            s16g = spool.tile([128, G], F32, tag="s16")
            nc.scalar.activation(s16g[:], lnssg[:], ACTF.Exp,
                                 bias=c_ln16[:], scale=-0.5)
            blg = spool.tile([128, G], F32, tag="bl")
            nc.vector.tensor_tensor(blg[:], s16g[:], lnssg[:], A.mult)

            lnxg = lpool.tile([128, G * 256], BF16, tag="lnx")
            nc.scalar.activation(lnxg[:], xg[:], ACTF.Ln, bias=c_tiny[:])
            zg = zpool.tile([128, G * 256], BF16, tag="z")
            nc.vector.tensor_tensor(zg[:], xg[:], lnxg[:], A.mult)

            for j in range(G):
                k = g * G + j
                sl = slice(j * 256, (j + 1) * 256)
                amat = apool.tile([128, C], BF16, tag="amat")
                nc.vector.tensor_scalar(amat[:], clst[:], labs[:, k:k + 1],
                                        s16g[:, j:j + 1], A.is_equal, A.mult)
                bmat = bpool.tile([128, C], BF16, tag="bmat")
                nc.vector.tensor_scalar(bmat[:], clst[:], labs[:, k:k + 1],
                                        blg[:, j:j + 1], A.is_equal, A.mult)
                st, sp = (k == 0), (k == T - 1)
                nc.tensor.matmul(psA[:], amat[:], xg[:, sl],
                                 start=st, stop=sp)
                nc.tensor.matmul(psZ[:], amat[:], zg[:, sl],
                                 start=st, stop=sp)
                nc.tensor.matmul(psB[:], bmat[:], xg[:, sl],
                                 start=st, stop=sp)

        accs = opool.tile([C, 768], F32)
        nc.vector.tensor_copy(accs[:, 0:256], psA[:])
        nc.vector.tensor_copy(accs[:, 256:512], psZ[:])
        nc.vector.tensor_copy(accs[:, 512:768], psB[:])
        nc.sync.dma_start(out[:], accs[:])
    nc.compile()
    return nc


def _get_nc():
    if "nc" not in _cache:
        _cache["nc"] = _build_nc()
    return _cache["nc"]


def kernel(logits_clean, logits_aug1, logits_aug2, labels):
    import os

    import ml_dtypes
    from concourse.bass_utils import run_bass_kernel_spmd

    BF = ml_dtypes.bfloat16
    x3 = np.concatenate(
        [np.asarray(logits_clean, dtype=np.float32),
         np.asarray(logits_aug1, dtype=np.float32),
         np.asarray(logits_aug2, dtype=np.float32)], axis=0)
    lab1 = np.asarray(labels).astype(np.int64)
    lab3 = np.concatenate([lab1, lab1, lab1])

    cls = np.ascontiguousarray(
        np.broadcast_to(np.arange(C, dtype=np.float32), (128, C))).astype(BF)
    in_maps = []
    for c in range(N_CORES):
        sl = slice(c * R, (c + 1) * R)
        xc = x3[sl].astype(BF).reshape(T, 128, D).transpose(1, 0, 2)
        in_maps.append({
            "xin": np.ascontiguousarray(xc).reshape(128, T * D),
            "labt": np.ascontiguousarray(
                lab3[sl].reshape(T, 128).T.astype(np.float32)),
            "cls": cls,
        })

    nc = _get_nc()
    trace = bool(int(os.environ.get("KERNEL_TRACE", "0")))
    kw = {}
    if trace:
        kw = dict(trace=True, tmpdir=os.environ.get("KERNEL_TRACE_DIR"))
    br = run_bass_kernel_spmd(nc, in_maps, list(range(N_CORES)), **kw)
    _cache["last_results"] = br

    acc = np.zeros((C, 768), np.float64)
    for c in range(N_CORES):
        acc += br.results[c]["acc"].astype(np.float64)

    seg = acc[:, 0:D] / 16.0                      # S_c per dim
    E = (acc[:, D:2 * D].sum(1) - 0.5 * acc[:, 2 * D:3 * D].sum(1)) / 16.0
    counts = np.bincount(lab3, minlength=C).astype(np.float64)
    mix = seg / np.maximum(counts, 1.0)[:, None]
    lm = np.log(np.maximum(mix, 1e-7))
    num = E - (seg * lm).sum(1)
    loss = np.where(counts > 0, num / np.maximum(counts, 1.0), 0.0).sum() / D
    return np.float32(0.01 * loss)


# revision 7
# speedup vs baseline: 1.3769x; 1.1187x over previous
"""SupJSD / ContrastiveLossPlus loss kernel for 8 Trainium2 NeuronCores.

Math: p_i = x_i/||x_i||, and with ln p = ln x - 0.5*ln(ss_i) (ss = sum x^2)
the loss needs only three per-class matrices, all PE matmuls:
  A_x[c,d] = sum_i 1hot*s16*x        (16*S_c, S_c = class prob sums)
  A_z[c,d] = sum_i 1hot*s16*z        (z = x*ln x)
  B[c,d]   = sum_i 1hot*(s16*lnss)*x
where sum_{i in c} sum_d p ln p = (sum_d A_z - 0.5*sum_d B)/16.

Per 8-tile group (128x2048 bf16):
  DMA   : x (partition-major contiguous) + host-built one-hot labels
          (class-major [128, 80, 8] per group).
  ACT   : Square(x) -> q, batched Ln(x) -> lnx, tiny Ln/Exp for
          s16 = 16/sqrt(ss). One pinned ln+exp+square table load.
  DVE   : fold-chain row-sum of q -> ss (bf16 tensor_tensor adds at 2x +
          one small reduce; avoids the slow cache-reduce/accum paths),
          z = x*lnx into the interleaved [x|z] tile, and the two
          stationary builds as single broadcast tensor_tensor mults.
  PE    : per tile one 512-col matmul (amat^T @ [x|z]) + one 256-col
          (bmat^T @ x), accumulating in separate PSUM banks.
Host combines the [80,768] per-core accumulators in float64.
"""

import numpy as np

N_CORES = 8
N, D, C = 65536, 256, 80
R = 3 * N // N_CORES          # rows per core = 24576
T = R // 128                  # tiles per core = 192
G = 8                         # tiles per group
NG = T // G                   # groups = 24
LOG16 = float(np.log(16.0))

_cache = {}


def _build_nc():
    from contextlib import ExitStack

    import concourse.tile as tile
    from concourse import bacc, mybir

    F32 = mybir.dt.float32
    BF16 = mybir.dt.bfloat16
    A = mybir.AluOpType
    ACTF = mybir.ActivationFunctionType
    AX = mybir.AxisListType

    nc = bacc.Bacc("TRN2", target_bir_lowering=False, debug=False,
                   num_devices=N_CORES)
    # Partition-major: xin[p, g*2048 + j*256 + c] = x[(g*8+j)*128 + p, c]
    xin = nc.dram_tensor("xin", [128, T * 256], BF16, kind="ExternalInput").ap()
    # One-hot labels, class-major per group: ohin[p, g*640 + c*8 + j]
    ohin = nc.dram_tensor("ohin", [128, NG * C * G], BF16,
                          kind="ExternalInput").ap()
    out = nc.dram_tensor("acc", [C, 768], F32, kind="ExternalOutput").ap()

    with tile.TileContext(nc) as tc, ExitStack() as ctx:
        cpool = ctx.enter_context(tc.tile_pool(name="consts", bufs=1))
        xzpool = ctx.enter_context(tc.tile_pool(name="xz", bufs=3))
        opool_h = ctx.enter_context(tc.tile_pool(name="oh", bufs=3))
        qpool = ctx.enter_context(tc.tile_pool(name="q", bufs=2))
        fpool = ctx.enter_context(tc.tile_pool(name="fold", bufs=2))
        lpool = ctx.enter_context(tc.tile_pool(name="lnx", bufs=2))
        spool = ctx.enter_context(tc.tile_pool(name="small", bufs=3))
        mpool = ctx.enter_context(tc.tile_pool(name="mats", bufs=2))
        opool = ctx.enter_context(tc.tile_pool(name="out", bufs=1))
        psa = ctx.enter_context(tc.tile_pool(name="psa", bufs=1, space="PSUM"))
        psb = ctx.enter_context(tc.tile_pool(name="psb", bufs=1, space="PSUM"))

        c_tiny = cpool.tile([128, 1], F32)
        nc.vector.memset(c_tiny[:], 1e-30)
        c_ln16 = cpool.tile([128, 1], F32)
        nc.vector.memset(c_ln16[:], LOG16)

        # Pin the ln+exp+square table once: avoids per-group table thrash.
        nc.scalar.add_instruction(mybir.InstLoadActFuncSet(
            name=nc.get_next_instruction_name(), act_func_set_id=6,
            ins=[], outs=[]))

        psA = psa.tile([C, 512], F32)
        psB = psb.tile([C, 256], F32)

        for g in range(NG):
            # [128, 8, 512]: cols 0:256 x (DMA), 256:512 z (TT)
            xzg = xzpool.tile([128, G, 512], BF16, tag="xz")
            nc.sync.dma_start(xzg[:, :, 0:256],
                              xin[:, g * G * 256:(g + 1) * G * 256]
                              .rearrange("p (t c) -> p t c", t=G))
            ohg = opool_h.tile([128, C, G], BF16, tag="oh")
            nc.sync.dma_start(ohg[:], ohin[:, g * C * G:(g + 1) * C * G]
                              .rearrange("p (c t) -> p c t", c=C))

            xv = xzg[:, :, 0:256]
            # q = x*x on ACT (batched Square), lnx batched Ln
            qg = qpool.tile([128, G * 256], BF16, tag="q")
            nc.scalar.activation(qg[:].rearrange("p (t c) -> p t c", t=G),
                                 xv, ACTF.Square)
            lnxg = lpool.tile([128, G * 256], BF16, tag="lnx")
            nc.scalar.activation(lnxg[:].rearrange("p (t c) -> p t c", t=G),
                                 xv, ACTF.Ln, bias=c_tiny[:])

            # fold-chain row sums of q: [8,256]->[8,128]->[8,64]->[8,32]->ss
            f1 = fpool.tile([128, G, 128], BF16, tag="f1")
            q3 = qg[:].rearrange("p (t c) -> p t c", t=G)
            nc.vector.tensor_tensor(f1[:], q3[:, :, 0:128], q3[:, :, 128:256],
                                    A.add)
            nc.vector.tensor_tensor(f1[:, :, 0:64], f1[:, :, 0:64],
                                    f1[:, :, 64:128], A.add)
            nc.vector.tensor_tensor(f1[:, :, 0:32], f1[:, :, 0:32],
                                    f1[:, :, 32:64], A.add)
            ssg = spool.tile([128, G], F32, tag="ss")
            nc.vector.tensor_reduce(ssg[:], f1[:, :, 0:32], AX.X, A.add)

            # z = x * lnx into the interleaved tile
            nc.vector.tensor_tensor(xzg[:, :, 256:512], xv,
                                    lnxg[:].rearrange("p (t c) -> p t c", t=G),
                                    A.mult)

            lnssg = spool.tile([128, G], F32, tag="lnss")
            nc.scalar.activation(lnssg[:], ssg[:], ACTF.Ln)
            s16g = spool.tile([128, G], F32, tag="s16")
            nc.scalar.activation(s16g[:], lnssg[:], ACTF.Exp,
                                 bias=c_ln16[:], scale=-0.5)
            s16b = spool.tile([128, G], BF16, tag="s16b")
            nc.vector.tensor_copy(s16b[:], s16g[:])
            blb = spool.tile([128, G], BF16, tag="blb")
            nc.vector.tensor_tensor(blb[:], s16g[:], lnssg[:], A.mult)

            # stationaries: amat[p,c,j] = oh*s16_j ; bmat[p,c,j] = oh*bl_j
            amat = mpool.tile([128, C, G], BF16, tag="amat")
            nc.vector.tensor_tensor(amat[:], ohg[:],
                                    s16b[:].rearrange("p (o t) -> p o t", o=1).broadcast_to([128, C, G]), A.mult)
            bmat = mpool.tile([128, C, G], BF16, tag="bmat")
            nc.vector.tensor_tensor(bmat[:], ohg[:],
                                    blb[:].rearrange("p (o t) -> p o t", o=1).broadcast_to([128, C, G]), A.mult)

            for j in range(G):
                k = g * G + j
                st, sp = (k == 0), (k == T - 1)
                nc.tensor.matmul(psA[:], amat[:, :, j:j + 1], xzg[:, j, :],
                                 start=st, stop=sp)
                nc.tensor.matmul(psB[:], bmat[:, :, j:j + 1],
                                 xzg[:, j, 0:256], start=st, stop=sp)

        accs = opool.tile([C, 768], F32)
        nc.vector.tensor_copy(accs[:, 0:512], psA[:])
        nc.vector.tensor_copy(accs[:, 512:768], psB[:])
        nc.sync.dma_start(out[:], accs[:])
    nc.compile()
    return nc


def _get_nc():
    if "nc" not in _cache:
        _cache["nc"] = _build_nc()
    return _cache["nc"]


def kernel(logits_clean, logits_aug1, logits_aug2, labels):
    import os

    import ml_dtypes
    from concourse.bass_utils import run_bass_kernel_spmd

    BF = ml_dtypes.bfloat16
    x3 = np.concatenate(
        [np.asarray(logits_clean, dtype=np.float32),
         np.asarray(logits_aug1, dtype=np.float32),
         np.asarray(logits_aug2, dtype=np.float32)], axis=0)
    lab1 = np.asarray(labels).astype(np.int64)
    lab3 = np.concatenate([lab1, lab1, lab1])

    cls_ar = np.arange(C, dtype=np.int64)
    in_maps = []
    for c in range(N_CORES):
        sl = slice(c * R, (c + 1) * R)
        xc = x3[sl].astype(BF).reshape(T, 128, D).transpose(1, 0, 2)
        # one-hot [128, NG, C, G]: oh[p, g, c, j] = (lab[(g*G+j)*128+p] == c)
        L = lab3[sl].reshape(NG, G, 128)
        oh = (L.transpose(2, 0, 1)[:, :, None, :] ==
              cls_ar[None, None, :, None]).astype(BF)
        in_maps.append({
            "xin": np.ascontiguousarray(xc).reshape(128, T * D),
            "ohin": np.ascontiguousarray(oh).reshape(128, NG * C * G),
        })

    nc = _get_nc()
    trace = bool(int(os.environ.get("KERNEL_TRACE", "0")))
    kw = {}
    if trace:
        kw = dict(trace=True, tmpdir=os.environ.get("KERNEL_TRACE_DIR"))
    br = run_bass_kernel_spmd(nc, in_maps, list(range(N_CORES)), **kw)
    _cache["last_results"] = br

    acc = np.zeros((C, 768), np.float64)
    for c in range(N_CORES):
        acc += br.results[c]["acc"].astype(np.float64)

    seg = acc[:, 0:D] / 16.0                      # S_c per dim
    E = (acc[:, D:2 * D].sum(1) - 0.5 * acc[:, 2 * D:3 * D].sum(1)) / 16.0
    counts = np.bincount(lab3, minlength=C).astype(np.float64)
    mix = seg / np.maximum(counts, 1.0)[:, None]
    lm = np.log(np.maximum(mix, 1e-7))
    num = E - (seg * lm).sum(1)
    loss = np.where(counts > 0, num / np.maximum(counts, 1.0), 0.0).sum() / D
    return np.float32(0.01 * loss)


# revision 8
# speedup vs baseline: 1.5484x; 1.1245x over previous
"""SupJSD / ContrastiveLossPlus loss kernel for 8 Trainium2 NeuronCores.

Math: p_i = x_i/||x_i||, and with ln p = ln x - 0.5*ln(ss_i) (ss = sum x^2)
the loss needs only three per-class matrices, all PE matmuls:
  A_x[c,d] = sum_i 1hot*s16*x        (16*S_c, S_c = class prob sums)
  A_z[c,d] = sum_i 1hot*s16*z        (z = x*ln x)
  B[c,d]   = sum_i 1hot*(s16*lnss)*x
where sum_{i in c} sum_d p ln p = (sum_d A_z - 0.5*sum_d B)/16.

Per 8-tile group (128x2048 bf16), software-pipelined one group deep so no
engine queue head-of-line blocks another group's work:
  stage BIG(g) : DMA x + one-hot, ACT Square(x)->q and batched Ln(x)->lnx,
                 DVE fold-chain row-sums of q -> ss, DVE z = x*lnx into the
                 interleaved [x|z] tile.
  stage FIN(g) : ACT tiny Ln/Exp for s16 = 16/sqrt(ss) (bf16 out), DVE
                 broadcast-mult stationary builds (tile-major, contiguous
                 LDWEIGHTS), PE per tile: 512-col matmul amat^T @ [x|z]
                 + 256-col bmat^T @ x into separate PSUM banks.
One pinned ln+exp+square activation table load. Host combines the
[80,768] per-core accumulators in float64.
"""

import numpy as np

N_CORES = 8
N, D, C = 65536, 256, 80
R = 3 * N // N_CORES          # rows per core = 24576
T = R // 128                  # tiles per core = 192
G = 8                         # tiles per group
NG = T // G                   # groups = 24
LOG16 = float(np.log(16.0))

_cache = {}


def _build_nc():
    from contextlib import ExitStack

    import concourse.tile as tile
    from concourse import bacc, mybir

    F32 = mybir.dt.float32
    BF16 = mybir.dt.bfloat16
    A = mybir.AluOpType
    ACTF = mybir.ActivationFunctionType
    AX = mybir.AxisListType

    nc = bacc.Bacc("TRN2", target_bir_lowering=False, debug=False,
                   num_devices=N_CORES)
    # Partition-major: xin[p, g*2048 + j*256 + c] = x[(g*8+j)*128 + p, c]
    xin = nc.dram_tensor("xin", [128, T * 256], BF16, kind="ExternalInput").ap()
    # One-hot labels, tile-major per group: ohin[p, g*640 + j*80 + c]
    ohin = nc.dram_tensor("ohin", [128, NG * G * C], BF16,
                          kind="ExternalInput").ap()
    out = nc.dram_tensor("acc", [C, 768], F32, kind="ExternalOutput").ap()

    with tile.TileContext(nc) as tc, ExitStack() as ctx:
        cpool = ctx.enter_context(tc.tile_pool(name="consts", bufs=1))
        xzpool = ctx.enter_context(tc.tile_pool(name="xz", bufs=4))
        ohpool = ctx.enter_context(tc.tile_pool(name="oh", bufs=4))
        qpool = ctx.enter_context(tc.tile_pool(name="q", bufs=2))
        fpool = ctx.enter_context(tc.tile_pool(name="fold", bufs=2))
        lpool = ctx.enter_context(tc.tile_pool(name="lnx", bufs=2))
        spool = ctx.enter_context(tc.tile_pool(name="small", bufs=3))
        mpool = ctx.enter_context(tc.tile_pool(name="mats", bufs=2))
        opool = ctx.enter_context(tc.tile_pool(name="out", bufs=1))
        psa = ctx.enter_context(tc.tile_pool(name="psa", bufs=1, space="PSUM"))
        psb = ctx.enter_context(tc.tile_pool(name="psb", bufs=1, space="PSUM"))

        c_tiny = cpool.tile([128, 1], F32)
        nc.vector.memset(c_tiny[:], 1e-30)
        c_ln16 = cpool.tile([128, 1], F32)
        nc.vector.memset(c_ln16[:], LOG16)

        # Pin the ln+exp+square table once: avoids per-group table thrash.
        nc.scalar.add_instruction(mybir.InstLoadActFuncSet(
            name=nc.get_next_instruction_name(), act_func_set_id=6,
            ins=[], outs=[]))

        psA = psa.tile([C, 512], F32)
        psB = psb.tile([C, 256], F32)

        state = {}

        def dma_stage(g):
            xzg = xzpool.tile([128, G, 512], BF16, tag="xz")
            nc.sync.dma_start(xzg[:, :, 0:256],
                              xin[:, g * G * 256:(g + 1) * G * 256]
                              .rearrange("p (t c) -> p t c", t=G))
            ohg = ohpool.tile([128, G, C], BF16, tag="oh")
            nc.sync.dma_start(ohg[:], ohin[:, g * G * C:(g + 1) * G * C]
                              .rearrange("p (t c) -> p t c", t=G))
            state[g] = {"xz": xzg, "oh": ohg}

        def big_stage(g):
            st = state[g]
            xzg = st["xz"]
            xv = xzg[:, :, 0:256]
            qg = qpool.tile([128, G * 256], BF16, tag="q")
            nc.scalar.activation(qg[:].rearrange("p (t c) -> p t c", t=G),
                                 xv, ACTF.Square)
            lnxg = lpool.tile([128, G * 256], BF16, tag="lnx")
            nc.scalar.activation(lnxg[:].rearrange("p (t c) -> p t c", t=G),
                                 xv, ACTF.Ln, bias=c_tiny[:])

            # fold-chain row sums of q: [8,256]->[8,128]->[8,64]->[8,32]->ss
            f1 = fpool.tile([128, G, 128], BF16, tag="f1")
            q3 = qg[:].rearrange("p (t c) -> p t c", t=G)
            nc.vector.tensor_tensor(f1[:], q3[:, :, 0:128], q3[:, :, 128:256],
                                    A.add)
            nc.vector.tensor_tensor(f1[:, :, 0:64], f1[:, :, 0:64],
                                    f1[:, :, 64:128], A.add)
            nc.vector.tensor_tensor(f1[:, :, 0:32], f1[:, :, 0:32],
                                    f1[:, :, 32:64], A.add)
            ssg = spool.tile([128, G], F32, tag="ss")
            nc.vector.tensor_reduce(ssg[:], f1[:, :, 0:32], AX.X, A.add)

            # z = x * lnx into the interleaved tile
            nc.vector.tensor_tensor(xzg[:, :, 256:512], xv,
                                    lnxg[:].rearrange("p (t c) -> p t c", t=G),
                                    A.mult)
            st["ss"] = ssg

        def fin_stage(g):
            st = state.pop(g)
            xzg, ohg, ssg = st["xz"], st["oh"], st["ss"]
            lnssg = spool.tile([128, G], F32, tag="lnss")
            nc.scalar.activation(lnssg[:], ssg[:], ACTF.Ln)
            s16b = spool.tile([128, G], BF16, tag="s16b")
            nc.scalar.activation(s16b[:], lnssg[:], ACTF.Exp,
                                 bias=c_ln16[:], scale=-0.5)
            blb = spool.tile([128, G], BF16, tag="blb")
            nc.vector.tensor_tensor(blb[:], s16b[:], lnssg[:], A.mult)

            # stationaries: amat[p,j,c] = oh*s16_j ; bmat[p,j,c] = oh*bl_j
            amat = mpool.tile([128, G, C], BF16, tag="amat")
            nc.vector.tensor_tensor(
                amat[:], ohg[:],
                s16b[:].rearrange("p (t o) -> p t o", o=1)
                .broadcast_to([128, G, C]), A.mult)
            bmat = mpool.tile([128, G, C], BF16, tag="bmat")
            nc.vector.tensor_tensor(
                bmat[:], ohg[:],
                blb[:].rearrange("p (t o) -> p t o", o=1)
                .broadcast_to([128, G, C]), A.mult)

            for j in range(G):
                k = g * G + j
                first, last = (k == 0), (k == T - 1)
                nc.tensor.matmul(psA[:], amat[:, j, :], xzg[:, j, :],
                                 start=first, stop=last)
                nc.tensor.matmul(psB[:], bmat[:, j, :], xzg[:, j, 0:256],
                                 start=first, stop=last)

        dma_stage(0)
        dma_stage(1)
        for g in range(NG):
            if g + 2 < NG:
                dma_stage(g + 2)
            big_stage(g)
            if g > 0:
                fin_stage(g - 1)
        fin_stage(NG - 1)

        accs = opool.tile([C, 768], F32)
        nc.vector.tensor_copy(accs[:, 0:512], psA[:])
        nc.vector.tensor_copy(accs[:, 512:768], psB[:])
        nc.sync.dma_start(out[:], accs[:])
    nc.compile()
    return nc


def _get_nc():
    if "nc" not in _cache:
        _cache["nc"] = _build_nc()
    return _cache["nc"]


def kernel(logits_clean, logits_aug1, logits_aug2, labels):
    import os

    import ml_dtypes
    from concourse.bass_utils import run_bass_kernel_spmd

    BF = ml_dtypes.bfloat16
    x3 = np.concatenate(
        [np.asarray(logits_clean, dtype=np.float32),
         np.asarray(logits_aug1, dtype=np.float32),
         np.asarray(logits_aug2, dtype=np.float32)], axis=0)
    lab1 = np.asarray(labels).astype(np.int64)
    lab3 = np.concatenate([lab1, lab1, lab1])

    cls_ar = np.arange(C, dtype=np.int64)
    in_maps = []
    for c in range(N_CORES):
        sl = slice(c * R, (c + 1) * R)
        xc = x3[sl].astype(BF).reshape(T, 128, D).transpose(1, 0, 2)
        # one-hot [128, NG, G, C]: oh[p, g, j, c] = (lab[(g*G+j)*128+p] == c)
        L = lab3[sl].reshape(NG, G, 128)
        oh = (L.transpose(2, 0, 1)[:, :, :, None] ==
              cls_ar[None, None, None, :]).astype(BF)
        in_maps.append({
            "xin": np.ascontiguousarray(xc).reshape(128, T * D),
            "ohin": np.ascontiguousarray(oh).reshape(128, NG * G * C),
        })

    nc = _get_nc()
    trace = bool(int(os.environ.get("KERNEL_TRACE", "0")))
    kw = {}
    if trace:
        kw = dict(trace=True, tmpdir=os.environ.get("KERNEL_TRACE_DIR"))
    br = run_bass_kernel_spmd(nc, in_maps, list(range(N_CORES)), **kw)
    _cache["last_results"] = br

    acc = np.zeros((C, 768), np.float64)
    for c in range(N_CORES):
        acc += br.results[c]["acc"].astype(np.float64)

    seg = acc[:, 0:D] / 16.0                      # S_c per dim
    E = (acc[:, D:2 * D].sum(1) - 0.5 * acc[:, 2 * D:3 * D].sum(1)) / 16.0
    counts = np.bincount(lab3, minlength=C).astype(np.float64)
    mix = seg / np.maximum(counts, 1.0)[:, None]
    lm = np.log(np.maximum(mix, 1e-7))
    num = E - (seg * lm).sum(1)
    loss = np.where(counts > 0, num / np.maximum(counts, 1.0), 0.0).sum() / D
    return np.float32(0.01 * loss)
